# revision 25
# baseline (speedup 1.0000x reference)
"""Trainium2 Bass kernel for nn_AttentionWithNoDeconvPos.

Model (B=32, N=512, D=512, H=8, DH=64):
    h = x[:,:,None] * W_emb + b_emb                  # [B,N,D]  rank-1 per neuron
    q,k,v = h @ W{q,k,v} + b                         # MHA, 8 heads
    o = softmax(q k^T / 8) v ; o = o @ Wo + bo
    x_atten = einsum('bnd,nd->bn', o, W_un) + b_un
    y = ((x_atten @ W1 + b1) @ W2 + b2) @ W3 + b3    # no activations
    y = bilinear_resize(y.reshape(8,32) -> (16,64)); min-max normalize

Algebraic folding done on host (float64):
    q = x * Aq + Cq with Aq = W_emb@Wq, Cq = b_emb@Wq + bq (same k, v)
      -> embedding + QKV projections collapse to rank-1 elementwise ops.
    x_atten[b,n] = sum_h <oraw_h[b,n,:], M_h[n,:]> / Z_h[b,n] + c[n]
      with M = W_un @ Wo^T (folds output projection + unEmbedding; oraw is the
      pre-normalization attention output, Z the softmax partition function)
    ff chain + bilinear upsample fold into a single matrix:
      y_up = x_atten @ Wfu + bfu, Wfu = (W1@W2@W3) @ U^T, U = kron(A16, A64)

Sharding: data-parallel over batch, 4 samples per core, 8 cores, no collectives.

Device dataflow per core (matmuls fp32r, PV in bf16):
    qT,kT built transposed via diag-matmul: psum = Aq_chunk^T @ diag(x) blocks
    sT[m,n] = kT_h^T qT_h  (scores transposed, contraction over the head dim)
    exp on ScalarE (scale=1/8; safe without max subtraction: |s/8| < 60)
    oT_aug = [v_h | 1]^T @ exp  (ones column yields the Z row for free)
    u rows via (oT * MT-with-ones-row) mult + one-hot column-sum matmuls
      -> UZ psum [64, 512]: rows 0-31 = per-(sample,head) u, rows 32-63 = Z
    x_atten = group-sum(U * recip(Z)) + c ; ff matmul ; min-max normalize
"""
import os
import sys

sys.path.insert(0, "/opt/trn_rl_repo")

import numpy as np
from contextlib import ExitStack

import concourse.bass as bass
import concourse.bacc as bacc
import concourse.tile as tile
from concourse import mybir
from concourse.bass_utils import run_bass_kernel_spmd

B, N, D, H = 32, 512, 512, 8
DH = D // H  # 64
NCORES = 8
BPC = B // NCORES  # 4 samples per core
NCH = N // 128
DCH = D // 128

F32 = mybir.dt.float32
F32R = mybir.dt.float32r
BF16 = mybir.dt.bfloat16
EXP = mybir.ActivationFunctionType.Exp
ALU = mybir.AluOpType

_cached_nc = None
last_results = None  # test harness reads exec_time_ns off this


def _bilin_matrix(n_in, n_out):
    A = np.zeros((n_out, n_in))
    for i in range(n_out):
        c = (i + 0.5) * n_in / n_out - 0.5
        f = int(np.floor(c))
        w = c - f
        for idx, wt in [(f, 1.0 - w), (f + 1, w)]:
            if 0 <= idx < n_in:
                A[i, idx] += wt
    A /= A.sum(axis=1, keepdims=True)
    return A


def _phases():
    # bisection helper: KPHASES=diag,attn,fin (default all), KNB=#samples
    p = os.environ.get("KPHASES", "diag,attn,fin").split(",")
    nb = int(os.environ.get("KNB", str(BPC)))
    return ("diag" in p), ("attn" in p), ("fin" in p), nb


def _build_nc():
    do_diag, do_attn, do_fin, nb = _phases()
    nc = bacc.Bacc(name="attn_kernel")

    # ---- DRAM I/O (per core) ----
    d_x = nc.dram_tensor("xp", [128, BPC * NCH], F32, kind="ExternalInput")
    d_D4 = nc.dram_tensor("D4", [128, NCH, BPC * 128], F32R, kind="ExternalInput")
    d_Aq = nc.dram_tensor("Aq", [128, NCH, D], F32R, kind="ExternalInput")
    d_Ak = nc.dram_tensor("Ak", [128, NCH, D], F32R, kind="ExternalInput")
    d_CqT = nc.dram_tensor("CqT", [128, DCH, N], F32, kind="ExternalInput")
    d_CkT = nc.dram_tensor("CkT", [128, DCH, N], F32, kind="ExternalInput")
    d_Avp = nc.dram_tensor("Avp", [128, NCH, H, DH + 1], BF16, kind="ExternalInput")
    d_Cvp = nc.dram_tensor("Cvp", [128, NCH, H, DH + 1], BF16, kind="ExternalInput")
    d_MT = nc.dram_tensor("MTh", [65, H, N], F32, kind="ExternalInput")
    d_oneh = nc.dram_tensor("oneh", [65, B, 2 * B], F32R, kind="ExternalInput")
    d_wd = nc.dram_tensor("wd", [65, N], F32R, kind="ExternalInput")
    d_gsel = nc.dram_tensor("gsel", [B, BPC], F32R, kind="ExternalInput")
    d_eye = nc.dram_tensor("eye4", [BPC, BPC], F32, kind="ExternalInput")
    d_c = nc.dram_tensor("cvec", [BPC, N], F32, kind="ExternalInput")
    d_bfu = nc.dram_tensor("bfu", [BPC, 1024], F32, kind="ExternalInput")
    d_Wfu = nc.dram_tensor("Wfu", [128, NCH, 1024], F32R, kind="ExternalInput")

    d_y = nc.dram_tensor("y_out", [BPC, 1024], F32, kind="ExternalOutput")
    d_xa = nc.dram_tensor("xa_out", [BPC, N], F32, kind="ExternalOutput")

    with ExitStack() as ctx:
        tc = ctx.enter_context(tile.TileContext(nc))
        consts = ctx.enter_context(tc.tile_pool(name="consts", bufs=1))
        fin = ctx.enter_context(tc.tile_pool(name="fin", bufs=1))
        ps_u = ctx.enter_context(tc.tile_pool(name="ps_u", bufs=1, space="PSUM"))

        def load(pool, name, dram, shape, dtype, eng=None):
            t = pool.tile(shape, dtype, tag=name)
            (eng or nc.sync).dma_start(t[:], dram[:])
            return t

        # diag-phase inputs first, split across the three DMA-capable rings
        dload_cm = tc.tile_pool(name="dload", bufs=1)
        dload = dload_cm.__enter__()
        Aq_sb = load(dload, "Aq", d_Aq, [128, NCH, D], F32R, nc.sync)
        D4_sb = load(dload, "D4", d_D4, [128, NCH, BPC * 128], F32R, nc.scalar)
        Ak_sb = load(dload, "Ak", d_Ak, [128, NCH, D], F32R, nc.sync)
        CqT_sb = load(dload, "CqT", d_CqT, [128, DCH, N], F32, nc.gpsimd)
        CkT_sb = load(dload, "CkT", d_CkT, [128, DCH, N], F32, nc.gpsimd)

        # remaining constants
        x_sb = load(consts, "xp", d_x, [128, BPC * NCH], F32, nc.scalar)
        Avp_sb = load(consts, "Avp", d_Avp, [128, NCH, H, DH + 1], BF16, nc.scalar)
        Cvp_sb = load(consts, "Cvp", d_Cvp, [128, NCH, H, DH + 1], BF16, nc.scalar)
        MT_sb = load(consts, "MTh", d_MT, [65, H, N], F32, nc.scalar)
        oneh_sb = load(consts, "oneh", d_oneh, [65, B, 2 * B], F32R, nc.scalar)
        gsel_sb = load(consts, "gsel", d_gsel, [B, BPC], F32R, nc.gpsimd)
        eye_sb = load(consts, "eye4", d_eye, [BPC, BPC], F32, nc.gpsimd)
        c_sb = load(consts, "cvec", d_c, [BPC, N], F32, nc.gpsimd)
        bfu_sb = load(consts, "bfu", d_bfu, [BPC, 1024], F32, nc.gpsimd)

        U_ps = ps_u.tile([2 * B, N], F32, tag="U")
        if not (do_attn and nb == BPC):
            # bisection: fill U/Z with finite values so finals are well defined
            wd_sb = load(consts, "wd", d_wd, [65, N], F32R)
            nc.tensor.matmul(
                U_ps[:], oneh_sb[:, 0, :], wd_sb[:], start=True, stop=True
            )

        qT_sb = fin.tile([128, DCH, BPC, N], F32R, tag="qT")
        kT_sb = fin.tile([128, DCH, BPC, N], F32R, tag="kT")

        # ---- fused diag + attention: emit diag(dc=j) right before the
        # attention work of head-pair j so the PE stream pipelines them ----
        with tc.tile_pool(name="persb", bufs=2) as persb, tc.tile_pool(
            name="vpool", bufs=BPC
        ) as vpool, tc.tile_pool(name="small", bufs=2) as small, tc.tile_pool(
            name="ps_dg", bufs=2, space="PSUM"
        ) as ps_dg, tc.tile_pool(
            name="ps_s", bufs=2, space="PSUM"
        ) as ps_s, tc.tile_pool(name="ps_o", bufs=1, space="PSUM") as ps_o:
            # v_aug (bf16) for every sample up front: v = x*Avp + Cvp
            v_tiles = []
            for b in range(BPC):
                v_sb = vpool.tile([128, NCH, H, DH + 1], BF16, tag="v")
                for mc in range(NCH):
                    nc.vector.scalar_tensor_tensor(
                        v_sb[:, mc, :, :],
                        Avp_sb[:, mc, :, :],
                        x_sb[:, b * NCH + mc : b * NCH + mc + 1],
                        Cvp_sb[:, mc, :, :],
                        op0=ALU.mult,
                        op1=ALU.add,
                    )
                v_tiles.append(v_sb)

            def emit_diag(dc):
                # diag matmuls for d-chunk dc (both tensors, all samples)
                for A_sb, CT_sb, out_sb in (
                    (Aq_sb, CqT_sb, qT_sb),
                    (Ak_sb, CkT_sb, kT_sb),
                ):
                    for nck in range(NCH):
                        p_dg = ps_dg.tile([128, BPC * 128], F32, tag="dg")
                        nc.tensor.matmul(
                            p_dg[:],
                            A_sb[:, nck, dc * 128 : (dc + 1) * 128],
                            D4_sb[:, nck, :],
                            start=True,
                            stop=True,
                        )
                        ct = CT_sb[:, dc, nck * 128 : (nck + 1) * 128]
                        ct_b = bass.AP(
                            tensor=ct.tensor,
                            offset=ct.offset,
                            ap=[list(ct.ap[0]), [0, BPC], list(ct.ap[1])],
                        )
                        nc.vector.tensor_add(
                            out_sb[:, dc, :, nck * 128 : (nck + 1) * 128],
                            p_dg[:].rearrange("p (b n) -> p b n", b=BPC),
                            ct_b,
                        )

            emit_diag(0)
            for j in range(H // 2):  # head pair j uses qT/kT d-chunk j
                for b in range(BPC):
                    v_sb = v_tiles[b]
                    exp_sb = persb.tile([128, NCH, 2, N], BF16, tag="exp")
                    for mc in range(NCH):
                        p_s = ps_s.tile([128, 2, N], F32, tag="s")
                        for hh in range(2):
                            lo = hh * 64
                            nc.tensor.matmul(
                                p_s[:, hh, :],
                                kT_sb[lo : lo + 64, j, b, mc * 128 : (mc + 1) * 128],
                                qT_sb[lo : lo + 64, j, b, :],
                                start=True,
                                stop=True,
                            )
                        nc.scalar.activation(
                            exp_sb[:, mc, :, :], p_s[:], EXP, scale=0.125
                        )

                    for hh in range(2):
                        h = 2 * j + hh
                        slot = b * H + h
                        p_o = ps_o.tile([65, N], F32, tag="o")
                        for mc in range(NCH):
                            nc.tensor.matmul(
                                p_o[:],
                                v_sb[:, mc, h, :],
                                exp_sb[:, mc, hh, :],
                                start=(mc == 0),
                                stop=(mc == NCH - 1),
                            )
                        w_sb = small.tile([65, N], F32R, tag="w")
                        nc.vector.tensor_mul(w_sb[:], p_o[:], MT_sb[:, h, :])
                        nc.tensor.matmul(
                            U_ps[:],
                            oneh_sb[:, slot, :],
                            w_sb[:],
                            start=(j == 0 and b == 0 and hh == 0),
                            stop=(j == H // 2 - 1 and b == BPC - 1 and hh == 1),
                        )
                    if b == 0 and j < H // 2 - 1:
                        # hide the next pair's diag burst behind this pair's exps
                        emit_diag(j + 1)

        dload_cm.__exit__(None, None, None)

        if do_fin:
            # ---- phase 3: x_atten, ff, normalize ----
            with tc.tile_pool(name="ps_f", bufs=1, space="PSUM") as ps_f, tc.tile_pool(
                name="fpool", bufs=1
            ) as fpool:
                Wfu_sb = load(fpool, "Wfu", d_Wfu, [128, NCH, 1024], F32R, nc.gpsimd)
                R_sb = fpool.tile([B, N], F32, tag="R")
                nc.vector.reciprocal(R_sb[:], U_ps[B : 2 * B, :])
                W2_sb = fpool.tile([B, N], F32R, tag="W2")
                nc.vector.tensor_mul(W2_sb[:], U_ps[0:B, :], R_sb[:])
                p_xa = ps_f.tile([BPC, N], F32, tag="xa")
                nc.tensor.matmul(
                    p_xa[:], gsel_sb[:], W2_sb[:], start=True, stop=True
                )
                xa_sb = fpool.tile([BPC, N], F32, tag="xa_sb")
                nc.vector.tensor_add(xa_sb[:], p_xa[:], c_sb[:])
                nc.sync.dma_start(d_xa[:], xa_sb[:])

                p_xaT = ps_f.tile([128, NCH, BPC], F32, tag="xaT")
                for cc in range(NCH):
                    nc.tensor.transpose(
                        p_xaT[:, cc, :],
                        xa_sb[:, cc * 128 : (cc + 1) * 128],
                        eye_sb[:],
                    )
                xaT_sb = fpool.tile([128, NCH, BPC], F32R, tag="xaT_sb")
                nc.vector.tensor_copy(xaT_sb[:], p_xaT[:])
                p_ff = ps_f.tile([BPC, 1024], F32, tag="ff")
                for cc in range(NCH):
                    for half in range(2):
                        nc.tensor.matmul(
                            p_ff[:, half * 512 : (half + 1) * 512],
                            xaT_sb[:, cc, :],
                            Wfu_sb[:, cc, half * 512 : (half + 1) * 512],
                            start=(cc == 0),
                            stop=(cc == NCH - 1),
                        )
                yup_sb = fpool.tile([BPC, 1024], F32, tag="yup")
                mn_sb = fpool.tile([BPC, 1], F32, tag="mn")
                mx_sb = fpool.tile([BPC, 1], F32, tag="mx")
                nc.vector.tensor_add(yup_sb[:], p_ff[:], bfu_sb[:])
                nc.vector.tensor_reduce(
                    mn_sb[:], yup_sb[:], axis=mybir.AxisListType.X, op=ALU.min
                )
                nc.vector.tensor_reduce(
                    mx_sb[:], yup_sb[:], axis=mybir.AxisListType.X, op=ALU.max
                )
                rng_sb = fpool.tile([BPC, 1], F32, tag="rng")
                nc.vector.scalar_tensor_tensor(
                    rng_sb[:], mx_sb[:], 1.0, mn_sb[:],
                    op0=ALU.mult, op1=ALU.subtract,
                )
                nc.vector.tensor_scalar_add(rng_sb[:], rng_sb[:], 1e-8)
                ri_sb = fpool.tile([BPC, 1], F32, tag="ri")
                nc.vector.reciprocal(ri_sb[:], rng_sb[:])
                y_sb = fpool.tile([BPC, 1024], F32, tag="y")
                nc.vector.tensor_scalar(
                    y_sb[:], yup_sb[:], mn_sb[:], ri_sb[:],
                    op0=ALU.subtract, op1=ALU.mult,
                )
                nc.sync.dma_start(d_y[:], y_sb[:])
        else:
            xa_sb = fpool.tile([BPC, N], F32, tag="xa_sb")
            nc.vector.tensor_copy(xa_sb[:], c_sb[:])
            nc.sync.dma_start(d_xa[:], xa_sb[:])
            y_sb = fpool.tile([BPC, 1024], F32, tag="y")
            nc.vector.tensor_copy(y_sb[:], bfu_sb[:])
            nc.sync.dma_start(d_y[:], y_sb[:])
            if do_diag:
                # touch qT/kT so the tiles have readers
                t_sb = fin.tile([128, 2], F32, tag="touch")
                nc.vector.tensor_add(
                    t_sb[:, 0:1], qT_sb[:, 0, 0, 0:1], kT_sb[:, 0, 0, 0:1]
                )

    nc.compile()
    return nc


def _host_fold(inputs):
    f8 = lambda a: np.asarray(a, dtype=np.float64)
    W_emb, b_emb = f8(inputs["W_emb"]), f8(inputs["b_emb"])
    Wq, bq = f8(inputs["Wq"]), f8(inputs["bq"])
    Wk, bk = f8(inputs["Wk"]), f8(inputs["bk"])
    Wv, bv = f8(inputs["Wv"]), f8(inputs["bv"])
    Wo, bo = f8(inputs["Wo"]), f8(inputs["bo"])
    W_un, b_un = f8(inputs["W_un"]), f8(inputs["b_un"])
    W1, b1 = f8(inputs["W1"]), f8(inputs["b1"])
    W2, b2 = f8(inputs["W2"]), f8(inputs["b2"])
    W3, b3 = f8(inputs["W3"]), f8(inputs["b3"])

    Aq, Cq = W_emb @ Wq, b_emb @ Wq + bq
    Ak, Ck = W_emb @ Wk, b_emb @ Wk + bk
    Av, Cv = W_emb @ Wv, b_emb @ Wv + bv
    M = W_un @ Wo.T                      # [N, D]
    c = W_un @ bo + b_un                 # [N]
    W123 = W1 @ W2 @ W3                  # [N, 256]
    bf_ = b1 @ W2 @ W3 + b2 @ W3 + b3    # [256]
    U = np.kron(_bilin_matrix(8, 16), _bilin_matrix(32, 64))  # [1024, 256]
    Wfu = W123 @ U.T                     # [N, 1024]
    bfu = U @ bf_                        # [1024]

    import ml_dtypes

    def chunk_pn(a, dtype=np.float32):  # [N, F...] -> [128, N//128, F...]
        a2 = a.reshape(NCH, 128, -1).transpose(1, 0, 2)
        return np.ascontiguousarray(a2).astype(dtype)

    Avp = np.zeros((N, H, DH + 1))
    Cvp = np.zeros((N, H, DH + 1))
    Avp[:, :, :DH] = Av.reshape(N, H, DH)
    Cvp[:, :, :DH] = Cv.reshape(N, H, DH)
    Cvp[:, :, DH] = 1.0

    oneh = np.zeros((65, B, 2 * B), dtype=np.float32)
    for j in range(B):
        oneh[0:64, j, j] = 1.0
        oneh[64, j, B + j] = 1.0
    gsel = np.zeros((B, BPC), dtype=np.float32)
    for s in range(BPC):
        gsel[s * H : (s + 1) * H, s] = 1.0

    return {
        "Aq": chunk_pn(Aq),
        "Ak": chunk_pn(Ak),
        "CqT": chunk_pn(Cq.T),
        "CkT": chunk_pn(Ck.T),
        "Avp": chunk_pn(Avp.reshape(N, -1), ml_dtypes.bfloat16).reshape(
            128, NCH, H, DH + 1
        ),
        "Cvp": chunk_pn(Cvp.reshape(N, -1), ml_dtypes.bfloat16).reshape(
            128, NCH, H, DH + 1
        ),
        "MTh": np.concatenate(
            [
                np.ascontiguousarray(
                    M.T.reshape(H, DH, N).transpose(1, 0, 2)
                ).astype(np.float32),
                np.ones((1, H, N), dtype=np.float32),
            ],
            axis=0,
        ),
        "oneh": oneh,
        "wd": np.ones((65, N), dtype=np.float32),
        "gsel": gsel,
        "eye4": np.eye(BPC, dtype=np.float32),
        "cvec": np.broadcast_to(c.astype(np.float32), (BPC, N)).copy(),
        "bfu": np.broadcast_to(bfu.astype(np.float32), (BPC, 1024)).copy(),
        "Wfu": chunk_pn(Wfu),
    }


def kernel(**inputs):
    global _cached_nc, last_results
    const_ins = _host_fold(inputs)
    x = np.asarray(inputs["x"], dtype=np.float32)  # [B, N]

    in_maps = []
    for core in range(NCORES):
        xc = x[core * BPC : (core + 1) * BPC]  # [BPC, N]
        xp = np.ascontiguousarray(
            xc.reshape(BPC, NCH, 128).transpose(2, 0, 1)
        ).astype(np.float32)
        D4 = np.zeros((128, NCH, BPC, 128), dtype=np.float32)
        for nck in range(NCH):
            for b in range(BPC):
                np.fill_diagonal(
                    D4[:, nck, b, :], xc[b, nck * 128 : (nck + 1) * 128]
                )
        m = dict(const_ins)
        m["xp"] = xp.reshape(128, BPC * NCH)
        m["D4"] = D4.reshape(128, NCH, BPC * 128)
        in_maps.append(m)

    if _cached_nc is None:
        _cached_nc = _build_nc()

    res = run_bass_kernel_spmd(_cached_nc, in_maps, core_ids=list(range(NCORES)))
    last_results = res

    y = np.concatenate([r["y_out"] for r in res.results], axis=0)  # [B, 1024]
    xa = np.concatenate([r["xa_out"] for r in res.results], axis=0)  # [B, N]
    y = y.reshape(B, 1, 16, 64).astype(np.float32)
    xa = xa.reshape(B, N, 1).astype(np.float32)
    return (y, xa)


# revision 26
# speedup vs baseline: 1.0783x; 1.0783x over previous
"""Trainium2 Bass kernel for nn_AttentionWithNoDeconvPos.

Model (B=32, N=512, D=512, H=8, DH=64):
    h = x[:,:,None] * W_emb + b_emb                  # [B,N,D]  rank-1 per neuron
    q,k,v = h @ W{q,k,v} + b                         # MHA, 8 heads
    o = softmax(q k^T / 8) v ; o = o @ Wo + bo
    x_atten = einsum('bnd,nd->bn', o, W_un) + b_un
    y = ((x_atten @ W1 + b1) @ W2 + b2) @ W3 + b3    # no activations
    y = bilinear_resize(y.reshape(8,32) -> (16,64)); min-max normalize

Algebraic folding done on host (float64):
    q = x * Aq + Cq with Aq = W_emb@Wq, Cq = b_emb@Wq + bq (same k, v)
      -> embedding + QKV projections collapse to rank-1 elementwise ops.
    x_atten[b,n] = sum_h <oraw_h[b,n,:], M_h[n,:]> / Z_h[b,n] + c[n]
      with M = W_un @ Wo^T (folds output projection + unEmbedding; oraw is the
      pre-normalization attention output, Z the softmax partition function)
    ff chain + bilinear upsample fold into a single matrix:
      y_up = x_atten @ Wfu + bfu, Wfu = (W1@W2@W3) @ U^T, U = kron(A16, A64)

Sharding: data-parallel over batch, 4 samples per core, 8 cores, no collectives.

Device dataflow per core (matmuls fp32r, PV in bf16):
    qT,kT built transposed via diag-matmul: psum = Aq_chunk^T @ diag(x) blocks
    sT[m,n] = kT_h^T qT_h  (scores transposed, contraction over the head dim)
    exp on ScalarE (scale=1/8; safe without max subtraction: |s/8| < 60)
    oT_aug = [v_h | 1]^T @ exp  (ones column yields the Z row for free)
    u rows via (oT * MT-with-ones-row) mult + one-hot column-sum matmuls
      -> UZ psum [64, 512]: rows 0-31 = per-(sample,head) u, rows 32-63 = Z
    x_atten = group-sum(U * recip(Z)) + c ; ff matmul ; min-max normalize
"""
import os
import sys

sys.path.insert(0, "/opt/trn_rl_repo")

import numpy as np
from contextlib import ExitStack

import concourse.bass as bass
import concourse.bacc as bacc
import concourse.tile as tile
from concourse import mybir
from concourse.bass_utils import run_bass_kernel_spmd

B, N, D, H = 32, 512, 512, 8
DH = D // H  # 64
NCORES = 8
BPC = B // NCORES  # 4 samples per core
NCH = N // 128
DCH = D // 128

F32 = mybir.dt.float32
F32R = mybir.dt.float32r
BF16 = mybir.dt.bfloat16
EXP = mybir.ActivationFunctionType.Exp
ALU = mybir.AluOpType

_cached_nc = None
last_results = None  # test harness reads exec_time_ns off this


def _bilin_matrix(n_in, n_out):
    A = np.zeros((n_out, n_in))
    for i in range(n_out):
        c = (i + 0.5) * n_in / n_out - 0.5
        f = int(np.floor(c))
        w = c - f
        for idx, wt in [(f, 1.0 - w), (f + 1, w)]:
            if 0 <= idx < n_in:
                A[i, idx] += wt
    A /= A.sum(axis=1, keepdims=True)
    return A


def _phases():
    # bisection helper: KPHASES=diag,attn,fin (default all), KNB=#samples
    p = os.environ.get("KPHASES", "diag,attn,fin").split(",")
    nb = int(os.environ.get("KNB", str(BPC)))
    return ("diag" in p), ("attn" in p), ("fin" in p), nb


def _build_nc():
    do_diag, do_attn, do_fin, nb = _phases()
    nc = bacc.Bacc(name="attn_kernel")

    # ---- DRAM I/O (per core) ----
    d_x = nc.dram_tensor("xp", [128, BPC * NCH], F32, kind="ExternalInput")
    d_D4 = nc.dram_tensor("D4", [128, NCH, BPC * 128], F32R, kind="ExternalInput")
    d_Aq = nc.dram_tensor("Aq", [128, NCH, D], F32R, kind="ExternalInput")
    d_Ak = nc.dram_tensor("Ak", [128, NCH, D], F32R, kind="ExternalInput")
    d_CqT = nc.dram_tensor("CqT", [128, DCH, N], F32, kind="ExternalInput")
    d_CkT = nc.dram_tensor("CkT", [128, DCH, N], F32, kind="ExternalInput")
    d_Avp = nc.dram_tensor("Avp", [128, NCH, H, DH + 1], BF16, kind="ExternalInput")
    d_Cvp = nc.dram_tensor("Cvp", [128, NCH, H, DH + 1], BF16, kind="ExternalInput")
    d_MT = nc.dram_tensor("MTh", [65, H, N], F32, kind="ExternalInput")
    d_oneh = nc.dram_tensor("oneh", [65, B, 2 * B], F32R, kind="ExternalInput")
    d_wd = nc.dram_tensor("wd", [65, N], F32R, kind="ExternalInput")
    d_gsel = nc.dram_tensor("gsel", [B, BPC], F32R, kind="ExternalInput")
    d_eye = nc.dram_tensor("eye4", [BPC, BPC], F32, kind="ExternalInput")
    d_c = nc.dram_tensor("cvec", [BPC, N], F32, kind="ExternalInput")
    d_bfu = nc.dram_tensor("bfu", [BPC, 1024], F32, kind="ExternalInput")
    d_Wfu = nc.dram_tensor("Wfu", [128, NCH, 1024], F32R, kind="ExternalInput")

    d_y = nc.dram_tensor("y_out", [BPC, 1024], F32, kind="ExternalOutput")
    d_xa = nc.dram_tensor("xa_out", [BPC, N], F32, kind="ExternalOutput")

    with ExitStack() as ctx:
        tc = ctx.enter_context(tile.TileContext(nc))
        consts = ctx.enter_context(tc.tile_pool(name="consts", bufs=1))
        fin = ctx.enter_context(tc.tile_pool(name="fin", bufs=1))
        ps_u = ctx.enter_context(tc.tile_pool(name="ps_u", bufs=1, space="PSUM"))

        def load(pool, name, dram, shape, dtype, eng=None):
            t = pool.tile(shape, dtype, tag=name)
            (eng or nc.sync).dma_start(t[:], dram[:])
            return t

        # diag-phase inputs first, split across the three DMA-capable rings
        dload_cm = tc.tile_pool(name="dload", bufs=1)
        dload = dload_cm.__enter__()
        Aq_sb = load(dload, "Aq", d_Aq, [128, NCH, D], F32R, nc.sync)
        D4_sb = load(dload, "D4", d_D4, [128, NCH, BPC * 128], F32R, nc.scalar)
        Ak_sb = load(dload, "Ak", d_Ak, [128, NCH, D], F32R, nc.sync)
        CqT_sb = load(dload, "CqT", d_CqT, [128, DCH, N], F32, nc.gpsimd)
        CkT_sb = load(dload, "CkT", d_CkT, [128, DCH, N], F32, nc.gpsimd)

        # remaining constants
        x_sb = load(consts, "xp", d_x, [128, BPC * NCH], F32, nc.scalar)
        Avp_sb = load(consts, "Avp", d_Avp, [128, NCH, H, DH + 1], BF16, nc.scalar)
        Cvp_sb = load(consts, "Cvp", d_Cvp, [128, NCH, H, DH + 1], BF16, nc.scalar)
        MT_sb = load(consts, "MTh", d_MT, [65, H, N], F32, nc.scalar)
        oneh_sb = load(consts, "oneh", d_oneh, [65, B, 2 * B], F32R, nc.scalar)
        gsel_sb = load(consts, "gsel", d_gsel, [B, BPC], F32R, nc.gpsimd)
        eye_sb = load(consts, "eye4", d_eye, [BPC, BPC], F32, nc.gpsimd)
        c_sb = load(consts, "cvec", d_c, [BPC, N], F32, nc.gpsimd)
        bfu_sb = load(consts, "bfu", d_bfu, [BPC, 1024], F32, nc.gpsimd)

        U_ps = ps_u.tile([2 * B, N], F32, tag="U")
        if not (do_attn and nb == BPC):
            # bisection: fill U/Z with finite values so finals are well defined
            wd_sb = load(consts, "wd", d_wd, [65, N], F32R)
            nc.tensor.matmul(
                U_ps[:], oneh_sb[:, 0, :], wd_sb[:], start=True, stop=True
            )

        qT_sb = fin.tile([128, DCH, BPC, N], F32R, tag="qT")
        kT_sb = fin.tile([128, DCH, BPC, N], F32R, tag="kT")

        # ---- fused diag + attention: emit diag(dc=j) right before the
        # attention work of head-pair j so the PE stream pipelines them ----
        with tc.tile_pool(name="persb", bufs=2) as persb, tc.tile_pool(
            name="vpool", bufs=BPC
        ) as vpool, tc.tile_pool(name="small", bufs=2) as small, tc.tile_pool(
            name="ps_dg", bufs=2, space="PSUM"
        ) as ps_dg, tc.tile_pool(
            name="ps_s", bufs=2, space="PSUM"
        ) as ps_s, tc.tile_pool(name="ps_o", bufs=1, space="PSUM") as ps_o:
            # v_aug (bf16) for every sample up front: v = x*Avp + Cvp
            v_tiles = []
            for b in range(BPC):
                v_sb = vpool.tile([128, NCH, H, DH + 1], BF16, tag="v")
                for mc in range(NCH):
                    nc.vector.scalar_tensor_tensor(
                        v_sb[:, mc, :, :],
                        Avp_sb[:, mc, :, :],
                        x_sb[:, b * NCH + mc : b * NCH + mc + 1],
                        Cvp_sb[:, mc, :, :],
                        op0=ALU.mult,
                        op1=ALU.add,
                    )
                v_tiles.append(v_sb)

            def emit_diag(dc):
                # diag matmuls for d-chunk dc (both tensors, all samples)
                for A_sb, CT_sb, out_sb in (
                    (Aq_sb, CqT_sb, qT_sb),
                    (Ak_sb, CkT_sb, kT_sb),
                ):
                    for nck in range(NCH):
                        p_dg = ps_dg.tile([128, BPC * 128], F32, tag="dg")
                        nc.tensor.matmul(
                            p_dg[:],
                            A_sb[:, nck, dc * 128 : (dc + 1) * 128],
                            D4_sb[:, nck, :],
                            start=True,
                            stop=True,
                        )
                        ct = CT_sb[:, dc, nck * 128 : (nck + 1) * 128]
                        ct_b = bass.AP(
                            tensor=ct.tensor,
                            offset=ct.offset,
                            ap=[list(ct.ap[0]), [0, BPC], list(ct.ap[1])],
                        )
                        nc.vector.tensor_add(
                            out_sb[:, dc, :, nck * 128 : (nck + 1) * 128],
                            p_dg[:].rearrange("p (b n) -> p b n", b=BPC),
                            ct_b,
                        )

            for j in range(H // 2):  # head pair j uses qT/kT d-chunk j
                emit_diag(j)
                for b in range(BPC):
                    v_sb = v_tiles[b]
                    exp_sb = persb.tile([128, NCH, 2, N], BF16, tag="exp")
                    for mc in range(NCH):
                        p_s = ps_s.tile([128, 2, N], F32, tag="s")
                        for hh in range(2):
                            lo = hh * 64
                            nc.tensor.matmul(
                                p_s[:, hh, :],
                                kT_sb[lo : lo + 64, j, b, mc * 128 : (mc + 1) * 128],
                                qT_sb[lo : lo + 64, j, b, :],
                                start=True,
                                stop=True,
                            )
                        nc.scalar.activation(
                            exp_sb[:, mc, :, :], p_s[:], EXP, scale=0.125
                        )

                    for hh in range(2):
                        h = 2 * j + hh
                        slot = b * H + h
                        p_o = ps_o.tile([65, N], F32, tag="o")
                        for mc in range(NCH):
                            nc.tensor.matmul(
                                p_o[:],
                                v_sb[:, mc, h, :],
                                exp_sb[:, mc, hh, :],
                                start=(mc == 0),
                                stop=(mc == NCH - 1),
                            )
                        w_sb = small.tile([65, N], F32R, tag="w")
                        nc.vector.tensor_mul(w_sb[:], p_o[:], MT_sb[:, h, :])
                        nc.tensor.matmul(
                            U_ps[:],
                            oneh_sb[:, slot, :],
                            w_sb[:],
                            start=(j == 0 and b == 0 and hh == 0),
                            stop=(j == H // 2 - 1 and b == BPC - 1 and hh == 1),
                        )

        dload_cm.__exit__(None, None, None)

        if do_fin:
            # ---- phase 3: x_atten, ff, normalize ----
            with tc.tile_pool(name="ps_f", bufs=1, space="PSUM") as ps_f, tc.tile_pool(
                name="fpool", bufs=1
            ) as fpool:
                Wfu_sb = load(fpool, "Wfu", d_Wfu, [128, NCH, 1024], F32R, nc.gpsimd)
                R_sb = fpool.tile([B, N], F32, tag="R")
                nc.vector.reciprocal(R_sb[:], U_ps[B : 2 * B, :])
                W2_sb = fpool.tile([B, N], F32R, tag="W2")
                nc.vector.tensor_mul(W2_sb[:], U_ps[0:B, :], R_sb[:])
                p_xa = ps_f.tile([BPC, N], F32, tag="xa")
                nc.tensor.matmul(
                    p_xa[:], gsel_sb[:], W2_sb[:], start=True, stop=True
                )
                xa_sb = fpool.tile([BPC, N], F32, tag="xa_sb")
                nc.vector.tensor_add(xa_sb[:], p_xa[:], c_sb[:])
                nc.sync.dma_start(d_xa[:], xa_sb[:])

                p_xaT = ps_f.tile([128, NCH, BPC], F32, tag="xaT")
                for cc in range(NCH):
                    nc.tensor.transpose(
                        p_xaT[:, cc, :],
                        xa_sb[:, cc * 128 : (cc + 1) * 128],
                        eye_sb[:],
                    )
                xaT_sb = fpool.tile([128, NCH, BPC], F32R, tag="xaT_sb")
                nc.vector.tensor_copy(xaT_sb[:], p_xaT[:])
                p_ff = ps_f.tile([BPC, 1024], F32, tag="ff")
                for cc in range(NCH):
                    for half in range(2):
                        nc.tensor.matmul(
                            p_ff[:, half * 512 : (half + 1) * 512],
                            xaT_sb[:, cc, :],
                            Wfu_sb[:, cc, half * 512 : (half + 1) * 512],
                            start=(cc == 0),
                            stop=(cc == NCH - 1),
                        )
                yup_sb = fpool.tile([BPC, 1024], F32, tag="yup")
                mn_sb = fpool.tile([BPC, 1], F32, tag="mn")
                mx_sb = fpool.tile([BPC, 1], F32, tag="mx")
                nc.vector.tensor_add(yup_sb[:], p_ff[:], bfu_sb[:])
                nc.vector.tensor_reduce(
                    mn_sb[:], yup_sb[:], axis=mybir.AxisListType.X, op=ALU.min
                )
                nc.vector.tensor_reduce(
                    mx_sb[:], yup_sb[:], axis=mybir.AxisListType.X, op=ALU.max
                )
                rng_sb = fpool.tile([BPC, 1], F32, tag="rng")
                nc.vector.scalar_tensor_tensor(
                    rng_sb[:], mx_sb[:], 1.0, mn_sb[:],
                    op0=ALU.mult, op1=ALU.subtract,
                )
                nc.vector.tensor_scalar_add(rng_sb[:], rng_sb[:], 1e-8)
                ri_sb = fpool.tile([BPC, 1], F32, tag="ri")
                nc.vector.reciprocal(ri_sb[:], rng_sb[:])
                y_sb = fpool.tile([BPC, 1024], F32, tag="y")
                nc.vector.tensor_scalar(
                    y_sb[:], yup_sb[:], mn_sb[:], ri_sb[:],
                    op0=ALU.subtract, op1=ALU.mult,
                )
                nc.sync.dma_start(d_y[:], y_sb[:])
        else:
            xa_sb = fpool.tile([BPC, N], F32, tag="xa_sb")
            nc.vector.tensor_copy(xa_sb[:], c_sb[:])
            nc.sync.dma_start(d_xa[:], xa_sb[:])
            y_sb = fpool.tile([BPC, 1024], F32, tag="y")
            nc.vector.tensor_copy(y_sb[:], bfu_sb[:])
            nc.sync.dma_start(d_y[:], y_sb[:])
            if do_diag:
                # touch qT/kT so the tiles have readers
                t_sb = fin.tile([128, 2], F32, tag="touch")
                nc.vector.tensor_add(
                    t_sb[:, 0:1], qT_sb[:, 0, 0, 0:1], kT_sb[:, 0, 0, 0:1]
                )

    nc.compile()
    return nc


def _host_fold(inputs):
    f8 = lambda a: np.asarray(a, dtype=np.float64)
    W_emb, b_emb = f8(inputs["W_emb"]), f8(inputs["b_emb"])
    Wq, bq = f8(inputs["Wq"]), f8(inputs["bq"])
    Wk, bk = f8(inputs["Wk"]), f8(inputs["bk"])
    Wv, bv = f8(inputs["Wv"]), f8(inputs["bv"])
    Wo, bo = f8(inputs["Wo"]), f8(inputs["bo"])
    W_un, b_un = f8(inputs["W_un"]), f8(inputs["b_un"])
    W1, b1 = f8(inputs["W1"]), f8(inputs["b1"])
    W2, b2 = f8(inputs["W2"]), f8(inputs["b2"])
    W3, b3 = f8(inputs["W3"]), f8(inputs["b3"])

    Aq, Cq = W_emb @ Wq, b_emb @ Wq + bq
    Ak, Ck = W_emb @ Wk, b_emb @ Wk + bk
    Av, Cv = W_emb @ Wv, b_emb @ Wv + bv
    M = W_un @ Wo.T                      # [N, D]
    c = W_un @ bo + b_un                 # [N]
    W123 = W1 @ W2 @ W3                  # [N, 256]
    bf_ = b1 @ W2 @ W3 + b2 @ W3 + b3    # [256]
    U = np.kron(_bilin_matrix(8, 16), _bilin_matrix(32, 64))  # [1024, 256]
    Wfu = W123 @ U.T                     # [N, 1024]
    bfu = U @ bf_                        # [1024]

    import ml_dtypes

    def chunk_pn(a, dtype=np.float32):  # [N, F...] -> [128, N//128, F...]
        a2 = a.reshape(NCH, 128, -1).transpose(1, 0, 2)
        return np.ascontiguousarray(a2).astype(dtype)

    Avp = np.zeros((N, H, DH + 1))
    Cvp = np.zeros((N, H, DH + 1))
    Avp[:, :, :DH] = Av.reshape(N, H, DH)
    Cvp[:, :, :DH] = Cv.reshape(N, H, DH)
    Cvp[:, :, DH] = 1.0

    oneh = np.zeros((65, B, 2 * B), dtype=np.float32)
    for j in range(B):
        oneh[0:64, j, j] = 1.0
        oneh[64, j, B + j] = 1.0
    gsel = np.zeros((B, BPC), dtype=np.float32)
    for s in range(BPC):
        gsel[s * H : (s + 1) * H, s] = 1.0

    return {
        "Aq": chunk_pn(Aq),
        "Ak": chunk_pn(Ak),
        "CqT": chunk_pn(Cq.T),
        "CkT": chunk_pn(Ck.T),
        "Avp": chunk_pn(Avp.reshape(N, -1), ml_dtypes.bfloat16).reshape(
            128, NCH, H, DH + 1
        ),
        "Cvp": chunk_pn(Cvp.reshape(N, -1), ml_dtypes.bfloat16).reshape(
            128, NCH, H, DH + 1
        ),
        "MTh": np.concatenate(
            [
                np.ascontiguousarray(
                    M.T.reshape(H, DH, N).transpose(1, 0, 2)
                ).astype(np.float32),
                np.ones((1, H, N), dtype=np.float32),
            ],
            axis=0,
        ),
        "oneh": oneh,
        "wd": np.ones((65, N), dtype=np.float32),
        "gsel": gsel,
        "eye4": np.eye(BPC, dtype=np.float32),
        "cvec": np.broadcast_to(c.astype(np.float32), (BPC, N)).copy(),
        "bfu": np.broadcast_to(bfu.astype(np.float32), (BPC, 1024)).copy(),
        "Wfu": chunk_pn(Wfu),
    }


def kernel(**inputs):
    global _cached_nc, last_results
    const_ins = _host_fold(inputs)
    x = np.asarray(inputs["x"], dtype=np.float32)  # [B, N]

    in_maps = []
    for core in range(NCORES):
        xc = x[core * BPC : (core + 1) * BPC]  # [BPC, N]
        xp = np.ascontiguousarray(
            xc.reshape(BPC, NCH, 128).transpose(2, 0, 1)
        ).astype(np.float32)
        D4 = np.zeros((128, NCH, BPC, 128), dtype=np.float32)
        for nck in range(NCH):
            for b in range(BPC):
                np.fill_diagonal(
                    D4[:, nck, b, :], xc[b, nck * 128 : (nck + 1) * 128]
                )
        m = dict(const_ins)
        m["xp"] = xp.reshape(128, BPC * NCH)
        m["D4"] = D4.reshape(128, NCH, BPC * 128)
        in_maps.append(m)

    if _cached_nc is None:
        _cached_nc = _build_nc()

    res = run_bass_kernel_spmd(_cached_nc, in_maps, core_ids=list(range(NCORES)))
    last_results = res

    y = np.concatenate([r["y_out"] for r in res.results], axis=0)  # [B, 1024]
    xa = np.concatenate([r["xa_out"] for r in res.results], axis=0)  # [B, N]
    y = y.reshape(B, 1, 16, 64).astype(np.float32)
    xa = xa.reshape(B, N, 1).astype(np.float32)
    return (y, xa)
